# revision 13
# baseline (speedup 1.0000x reference)
"""Trainium2 Bass kernel for conv->BN->ReLU->1x1->ReLU->1x1->segment-mean classifier.

Contract: kernel(**inputs) takes FULL inputs (as from setup_inputs()) and
returns the FULL [4,16,512,512] float32 output. Internally shards across 8
NeuronCores: core = sample*2 + row_half (data-parallel over batch and H).

Device computes, per core (half-sample, 256 rows x 512 cols):
  feat = relu(conv3x3(x, w1*bn_inv) + bn_bias)      via K=28 im2col matmul
  h    = relu(w2 @ feat + b2)                        via K=256 matmul
  pooled_sums[128ch, 16*32 segments] = sum over 16x16 blocks of h
Host applies: /256, w3 @ . + b3, then the nearest "upsample" (row 2h broadcast).
"""
import sys, types
sys.path.insert(0, '/opt/trn_rl_repo')

import numpy as np
import ml_dtypes
from contextlib import ExitStack

import concourse.bass as bass
import concourse.tile as tile
from concourse import bacc, mybir
from concourse.alu_op_type import AluOpType
from concourse.bass_utils import run_bass_kernel_spmd

EPS = 1e-5
N_CORES = 8
MM_DT = mybir.dt.bfloat16  # full-rate PE path (fp32r runs cold at 2cyc/row)


def _build_program():
    nc = bacc.Bacc("TRN2", num_devices=N_CORES, debug=False,
                   target_bir_lowering=False)
    f32 = mybir.dt.float32
    xs = nc.dram_tensor("xs", [4, 258, 514], MM_DT, kind="ExternalInput")
    w1t = nc.dram_tensor("w1t", [64, 128], MM_DT, kind="ExternalInput")
    w2t = nc.dram_tensor("w2t", [128, 256], MM_DT, kind="ExternalInput")
    b2v = nc.dram_tensor("b2v", [128, 1], f32, kind="ExternalInput")
    pooled = nc.dram_tensor("pooled", [128, 512], f32, kind="ExternalOutput")

    Relu = mybir.ActivationFunctionType.Relu
    add, mx = AluOpType.add, AluOpType.max

    with tile.TileContext(nc) as tc:
        with ExitStack() as ctx:
            consts = ctx.enter_context(tc.tile_pool(name="consts", bufs=1))
            patch_pool = ctx.enter_context(tc.tile_pool(name="patch", bufs=4))
            pconv_pool = ctx.enter_context(
                tc.tile_pool(name="pconv", bufs=3, space="PSUM"))
            feat_pool = ctx.enter_context(tc.tile_pool(name="feat", bufs=8))
            ph_pool = ctx.enter_context(
                tc.tile_pool(name="ph", bufs=2, space="PSUM"))
            h_pool = ctx.enter_context(tc.tile_pool(name="h", bufs=6))
            hacc_pool = ctx.enter_context(tc.tile_pool(name="hacc", bufs=3))

            w1t_sb = consts.tile([64, 128], MM_DT)
            nc.sync.dma_start(w1t_sb[:], w1t.ap())
            w2t_sb = consts.tile([128, 256], MM_DT)
            nc.sync.dma_start(w2t_sb[:], w2t.ap())
            b2_sb = consts.tile([128, 1], f32)
            nc.sync.dma_start(b2_sb[:], b2v.ap())
            pooled_sb = consts.tile([128, 512], f32)

            xap = xs.ap()
            for t in range(16):
                patch = patch_pool.tile([64, 8192], MM_DT)
                pview = patch[:].rearrange("p (r c) -> p r c", r=16)
                for dy in range(3):
                    for dx in range(3):
                        k0 = (dy * 3 + dx) * 3
                        src = xap[0:3, 16 * t + dy:16 * t + dy + 16,
                                  dx:dx + 512]
                        nc.sync.dma_start(pview[k0:k0 + 3], src)
                        nc.gpsimd.dma_start(pview[32 + k0:32 + k0 + 3], src)
                src1 = xap[3:4, 16 * t:16 * t + 16, 0:512]
                nc.sync.dma_start(pview[27:28], src1)
                nc.gpsimd.dma_start(pview[59:60], src1)

                hacc = hacc_pool.tile([128, 512], MM_DT)
                for r in range(16):
                    c0 = r * 512
                    pc = pconv_pool.tile([128, 1024], f32)
                    nc.tensor.matmul(pc[:, 0:512],
                                     w1t_sb[0:28, :],
                                     patch[0:28, c0:c0 + 512],
                                     start=True, stop=True,
                                     tile_position=(0, 0))
                    nc.tensor.matmul(pc[:, 512:1024],
                                     w1t_sb[32:60, :],
                                     patch[32:60, c0:c0 + 512],
                                     start=True, stop=True,
                                     tile_position=(32, 0))
                    feat = feat_pool.tile([128, 1024], MM_DT)
                    nc.scalar.activation(feat[:], pc[:], Relu)
                    ph = ph_pool.tile([128, 512], f32)
                    nc.tensor.matmul(ph[:], w2t_sb[:, 0:128],
                                     feat[:, 0:512],
                                     start=True, stop=False)
                    nc.tensor.matmul(ph[:], w2t_sb[:, 128:256],
                                     feat[:, 512:1024],
                                     start=False, stop=True)
                    if r == 0:
                        nc.vector.tensor_scalar(hacc[:], ph[:], b2_sb[:],
                                                0.0, add, mx)
                    else:
                        h = h_pool.tile([128, 512], MM_DT)
                        nc.vector.tensor_scalar(h[:], ph[:], b2_sb[:],
                                                0.0, add, mx)
                        nc.vector.tensor_tensor(hacc[:], hacc[:], h[:], add)
                v = hacc[:].rearrange("p (g k) -> p g k", k=16)
                for half in (8, 4, 2, 1):
                    nc.vector.tensor_tensor(v[:, :, 0:half], v[:, :, 0:half],
                                            v[:, :, half:2 * half], add)
                nc.vector.tensor_copy(pooled_sb[:, t * 32:(t + 1) * 32],
                                      v[:, :, 0])
            nc.sync.dma_start(pooled.ap(), pooled_sb[:])
    nc.compile()
    return nc


_NC_CACHE = None


def _get_program():
    global _NC_CACHE
    if _NC_CACHE is None:
        _NC_CACHE = _build_program()
    return _NC_CACHE


def _run_device(in_maps, trace=False):
    nc = _get_program()
    if trace:
        import trn_agent_boot.trn_boot as _tb
        _hook = _tb._ntff_profile_via_ctypes('/opt/axon/libaxon_pjrt.so')
        _m = types.ModuleType('antenv.axon_hooks')
        _m.get_axon_ntff_profile_hook = lambda: _hook
        sys.modules['antenv.axon_hooks'] = _m
    return run_bass_kernel_spmd(nc, in_maps, list(range(N_CORES)), trace=trace)


def _prep_inputs(x, w1, b1, bn_gamma, bn_beta, bn_mean, bn_var, w2, b2):
    x = np.asarray(x, np.float32)
    inv = (bn_gamma / np.sqrt(bn_var + EPS)).astype(np.float32)
    w1f = (np.asarray(w1, np.float32) * inv[:, None, None, None])
    bias1 = (b1 * inv + bn_beta - bn_mean * inv).astype(np.float32)

    w1t_np = np.zeros((64, 128), np.float32)
    for dy in range(3):
        for dx in range(3):
            for ci in range(3):
                k = (dy * 3 + dx) * 3 + ci
                w1t_np[k, :] = w1f[0:128, ci, dy, dx]
                w1t_np[32 + k, :] = w1f[128:256, ci, dy, dx]
    w1t_np[27, :] = bias1[0:128]
    w1t_np[59, :] = bias1[128:256]

    w2t_np = np.zeros((128, 256), np.float32)
    w2t_np[:, 0:128] = np.asarray(w2, np.float32)[:, 0:128].T
    w2t_np[:, 128:256] = np.asarray(w2, np.float32)[:, 128:256].T
    b2_np = np.asarray(b2, np.float32).reshape(128, 1)

    w1t_bf = w1t_np.astype(ml_dtypes.bfloat16)
    w2t_bf = w2t_np.astype(ml_dtypes.bfloat16)
    xp = np.pad(x, ((0, 0), (0, 0), (1, 1), (1, 1)))  # [4,3,514,514]
    in_maps = []
    for core in range(N_CORES):
        b, half = core // 2, core % 2
        sh = np.empty((4, 258, 514), np.float32)
        sh[0:3] = xp[b, :, half * 256:half * 256 + 258, :]
        sh[3] = 1.0
        in_maps.append({"xs": sh.astype(ml_dtypes.bfloat16), "w1t": w1t_bf,
                        "w2t": w2t_bf, "b2v": b2_np})
    return in_maps


def _postprocess(results, w3, b3):
    w3 = np.asarray(w3, np.float32)
    b3 = np.asarray(b3, np.float32)
    out = np.empty((4, 16, 512, 512), np.float32)
    row_seg = (np.arange(512) * 1024) // 512  # = 2h
    for b in range(4):
        hs = np.concatenate(
            [results[2 * b]["pooled"], results[2 * b + 1]["pooled"]], axis=1)
        pooled_mean = hs / 256.0  # [128ch, 1024 segments]
        logits = w3 @ pooled_mean + b3[:, None]  # [16, 1024]
        out[b] = np.repeat(logits[:, row_seg, None], 512, axis=2)
    return out


def kernel(x, w1, b1, bn_gamma, bn_beta, bn_mean, bn_var, w2, b2, w3, b3,
           _trace=False):
    in_maps = _prep_inputs(x, w1, b1, bn_gamma, bn_beta, bn_mean, bn_var,
                           w2, b2)
    res = _run_device(in_maps, trace=_trace)
    out = _postprocess(res.results, w3, b3)
    if _trace:
        kernel.last_exec_time_ns = res.exec_time_ns
        kernel.last_results = res
    return out
